# revision 4
# baseline (speedup 1.0000x reference)
"""Trainium2 Bass kernel for nn_DecoderRNN: 64-step 2-layer tanh RNN + per-step FC.

Sharding (8 cores, no collectives):
  - 2-way data parallel over batch (cores 0-3: rows 0:128, cores 4-7: rows 128:256).
    Each group of 4 cores redundantly computes its batch-half's RNN.
  - 4-way tensor parallel over the fc output dim (O=8192 -> 2048 per core).

Numerics: everything on the PE runs in fp16 (state, RNN weights, FC weights).
fp16 keeps 10 mantissa bits (eps 4.9e-4) and tanh keeps |h|<=1, so the
recurrent quantization noise stays ~1.2e-2 total (vs 2e-2 gate; bf16 would
be ~7e-2 and fails). fp16 products accumulate exactly into fp32 PSUM.
Why fp16 over the baseline f32r: transposes run 1.0 cycles/row instead of
1.5, DVE ops get 2x throughput, and weight DMA bytes halve.

Per-core compute, per step t (B=128, H=1024, O_slice=2048):
  - State kept transposed ("g" form, [H, B]): g tiles are the matmul stationary
    operand (lhsT), weights stream as the moving operand at N=512.
  - RNN bias is added in-place on PSUM by the DVE (broadcast bias tile) after
    each 512-wide accumulation group, off the PE array. tanh on the scalar
    engine -> fp16 h_sb.
  - Layer outputs ([B, H] in PSUM) are transposed back to g form on the tensor
    engine (fp16 transposes), drained by DVE copies; the same fp16 g_h1 tile
    feeds both the next step's RNN and the FC (no separate cast).
  - FC bias is added during the DVE PSUM drain (broadcast bias tile), and FC
    matmul chunks are interleaved into the tanh/transpose latency gaps of the
    NEXT step to keep the PE array busy.
  - Startup: RNN weights stream on the sync DMA queue in first-needed order
    (layer-0 n-halves first), FC weights on the scalar-engine DMA queue in
    parallel, so step-0 matmuls start ~10us in instead of waiting ~60us for
    the whole weight set.
"""
import sys

sys.path.insert(0, "/opt/trn_rl_repo")

from contextlib import ExitStack

import numpy as np

import concourse.bass as bass
import concourse.tile as tile
from concourse import bacc, mybir
from concourse.bass_utils import run_bass_kernel_spmd

H = 1024
O = 8192
L = 2
T = 64
B = 256
N_CORES = 8
BG = B // 2          # batch rows per core (2-way DP)
OS = O // 4          # fc output slice per core (4-way TP)
KT = H // 128        # 8 k-tiles per 1024 contraction
F32 = mybir.dt.float32
F16 = mybir.dt.float16

_cached = {}

RNN_W_NAMES = ["ih0", "hh0", "ih1", "hh1"]


def _build_program(n_steps: int):
    nc = bacc.Bacc("TRN2", target_bir_lowering=False, debug=False, num_devices=N_CORES)

    # --- DRAM parameters (per-core shards, host-prepared layouts) ---
    # RNN weights, transposed+tiled on host: [p][k][n] of W.T, fp16
    wd = {}
    for nm in RNN_W_NAMES:
        wd[nm] = nc.declare_dram_parameter(f"w_{nm}", [128, KT, H], F16,
                                           isOutput=False)
    w_fc = nc.declare_dram_parameter("w_fc", [128, KT, OS], F16, isOutput=False)
    # initial state, g form: [p][k][b] = state[b, k*128+p], fp16
    gd = {}
    for nm in ("x", "h0", "h1"):
        gd[nm] = nc.declare_dram_parameter(f"g_{nm}", [128, KT, BG], F16,
                                           isOutput=False)
    # bias bcast tiles (b_ih + b_hh per layer), fc bias bcast, identity
    bd = {}
    for l in range(L):
        bd[f"b{l}"] = nc.declare_dram_parameter(f"b{l}", [128, H], F16,
                                                isOutput=False)
    fcbd = nc.declare_dram_parameter("fcb", [128, OS], F16, isOutput=False)
    identd = nc.declare_dram_parameter("ident", [128, 128], F16, isOutput=False)

    out_d = nc.declare_dram_parameter("out", [n_steps, 128, OS], F32, isOutput=True)

    with tile.TileContext(nc) as tc, ExitStack() as ctx:
        wpool = ctx.enter_context(tc.tile_pool(name="w", bufs=1))
        cpool = ctx.enter_context(tc.tile_pool(name="c", bufs=1))
        gp = ctx.enter_context(tc.tile_pool(name="gp", bufs=3))
        hp = ctx.enter_context(tc.tile_pool(name="h", bufs=2))
        logp = ctx.enter_context(tc.tile_pool(name="log", bufs=3))
        rnn_ps = ctx.enter_context(tc.tile_pool(name="rnnps", bufs=1, space="PSUM"))
        tr_ps = ctx.enter_context(tc.tile_pool(name="trps", bufs=2, space="PSUM"))
        fc_ps = ctx.enter_context(tc.tile_pool(name="fcps", bufs=2, space="PSUM"))

        # --- preamble: load constants + weights in first-needed order,
        # split across BOTH HW DGE queues (sync + scalar) so layer 0's first
        # 512-col group lands ~5us in: sync streams the ih weights, scalar
        # takes the small state/bias tiles then the hh weights; the FC weights
        # trail on both queues (not needed until step 1).
        g_tiles = {}
        w = {}
        for nm in RNN_W_NAMES:
            w[nm] = wpool.tile([128, KT, H], F16, tag=f"w{nm}", name=f"w{nm}")
        for nm in ("x", "h0", "h1"):
            g_tiles[nm] = gp.tile([128, KT, BG], F16, tag="g", name="g")
        ident = cpool.tile([128, 128], F16, tag="ident")
        bb = {}
        for nm in bd:
            bb[nm] = cpool.tile([128, H], F16, tag=nm, name=nm)
        fcb = cpool.tile([128, OS], F16, tag="fcb")
        wfc = wpool.tile([128, KT, OS], F16, tag="wfc")

        for nck in range(2):
            nsl = bass.ts(nck, 512)
            nc.sync.dma_start(w["ih0"][:, :, nsl], wd["ih0"][:, :, nsl])
        nc.scalar.dma_start(g_tiles["x"][:], gd["x"][:])
        nc.scalar.dma_start(g_tiles["h0"][:], gd["h0"][:])
        nc.scalar.dma_start(ident[:], identd[:])
        nc.scalar.dma_start(bb["b0"][:], bd["b0"][:])
        for nck in range(2):
            nsl = bass.ts(nck, 512)
            nc.scalar.dma_start(w["hh0"][:, :, nsl], wd["hh0"][:, :, nsl])
        nc.sync.dma_start(g_tiles["h1"][:], gd["h1"][:])
        nc.sync.dma_start(bb["b1"][:], bd["b1"][:])
        for nck in range(2):
            nsl = bass.ts(nck, 512)
            nc.sync.dma_start(w["ih1"][:, :, nsl], wd["ih1"][:, :, nsl])
            nc.scalar.dma_start(w["hh1"][:, :, nsl], wd["hh1"][:, :, nsl])
        nc.sync.dma_start(fcb[:], fcbd[:])
        for ci in range(4):
            fsl = bass.ts(ci, 512)
            eng = nc.sync if ci % 2 == 0 else nc.scalar
            eng.dma_start(wfc[:, :, fsl], w_fc[:, :, fsl])

        g_x = g_tiles["x"]
        g_h0 = g_tiles["h0"]
        g_h1 = g_tiles["h1"]

        def rnn_layer(g_in, g_h, w_in, w_h, b):
            """tanh(in @ W_ihT + h @ W_hhT + b) -> h_sb [128(B), H] fp16.

            Bias is added in-place on the DVE (PSUM += bias bcast tile), off
            the PE array; tanh on the scalar engine."""
            ps = rnn_ps.tile([128, H], F32, tag="rnnps")
            h_sb = hp.tile([128, H], F16, tag="h")
            for nck in range(2):
                nsl = bass.ts(nck, 512)
                for pi, (lhs, rhs) in enumerate(((g_in, w_in), (g_h, w_h))):
                    for k in range(KT):
                        first = pi == 0 and k == 0
                        last = pi == 1 and k == KT - 1
                        nc.tensor.matmul(ps[:, nsl], lhs[:, k, :], rhs[:, k, nsl],
                                         start=first, stop=last)
                nc.vector.tensor_add(ps[:, nsl], ps[:, nsl], b[:, nsl])
                # per-half tanh: the first half runs on the scalar engine
                # while the PE is still on the second half's matmuls
                nc.scalar.activation(h_sb[:, nsl], ps[:, nsl],
                                     mybir.ActivationFunctionType.Tanh)
            return h_sb

        def to_g(h_sb):
            """Transpose [B, H] -> g form [H(p), B] via REGULAR matmuls
            against the identity (out = h_sb_slice.T @ I).

            Regular matmuls pipeline back-to-back through the PE reorder
            window (~100ns each at N=128), unlike transpose-mode whose
            ~170ns fixed SBUF-access latency doesn't overlap. Numerically
            identical: fp16 h values pass through f32 PSUM exactly.
            4 transposed tiles per PSUM bank; each bank drained (and cast
            back to fp16) by a wide DVE copy."""
            g = gp.tile([128, KT, BG], F16, tag="g", name="g")
            for grp in range(2):
                pt = tr_ps.tile([128, 512], F32, tag="trps", name="pt")
                for j in range(4):
                    k = grp * 4 + j
                    nc.tensor.matmul(pt[:, bass.ts(j, 128)],
                                     h_sb[:, bass.ts(k, 128)], ident[:],
                                     start=True, stop=True)
                gs = g[:, grp * 4:(grp + 1) * 4, :]
                nc.vector.tensor_copy(gs, pt[:])
            return g

        def emit_fc_chunk(gb, tprev, ci):
            """FC chunk: logits[:, ci*512:(ci+1)*512] for step tprev (fp16)."""
            ps = fc_ps.tile([128, 512], F32, tag="fcps", name="fps")
            fsl = bass.ts(ci, 512)
            for k in range(KT):
                nc.tensor.matmul(ps[:], gb[:, k, :], wfc[:, k, fsl],
                                 start=(k == 0), stop=(k == KT - 1))
            lsb = logp.tile([128, 512], F32, tag="log", name="lsb")
            nc.vector.tensor_add(lsb[:], ps[:], fcb[:, fsl])
            nc.sync.dma_start(out_d[tprev][:, fsl], lsb[:])

        # Software pipeline: FC of step t-1 is interleaved into step t's
        # tanh/transpose gaps. pending = (g_h1, t_index) awaiting FC.
        pending = (g_h1, None)  # g from init; no FC for it

        for t in range(n_steps):
            gb_prev, tprev = pending
            h0_sb = rnn_layer(g_x, g_h0, w["ih0"], w["hh0"], bb["b0"])
            if tprev is not None:
                emit_fc_chunk(gb_prev, tprev, 0)
            g_h0 = to_g(h0_sb)
            if tprev is not None:
                emit_fc_chunk(gb_prev, tprev, 1)
            h1_sb = rnn_layer(g_h0, g_h1, w["ih1"], w["hh1"], bb["b1"])
            if tprev is not None:
                emit_fc_chunk(gb_prev, tprev, 2)
            g_h1 = to_g(h1_sb)
            if tprev is not None:
                emit_fc_chunk(gb_prev, tprev, 3)
            g_x = g_h1
            pending = (g_h1, t)

        # drain the last step's FC
        gb_prev, tprev = pending
        if tprev is not None:
            for ci in range(4):
                emit_fc_chunk(gb_prev, tprev, ci)

    nc.finalize()
    return nc


def _prep_inputs(x, hidden, W_ih, W_hh, b_ih, b_hh, fc_W, fc_b, n_steps):
    """Build the 8 per-core input maps (host-side transposes, fp16)."""
    def gform(a):  # [BG, H] -> [128, KT, BG]: out[p, k, b] = a[b, k*128+p]
        return np.ascontiguousarray(
            a.T.reshape(KT, 128, BG).transpose(1, 0, 2)).astype(np.float16)

    def wform(Wmat):  # [H_out, H_in] -> [128, KT, H_out] of W.T (fp16)
        return np.ascontiguousarray(
            Wmat.T.reshape(KT, 128, Wmat.shape[0]).transpose(1, 0, 2)).astype(
                np.float16)

    ident = np.eye(128, dtype=np.float16)

    common = {"ident": ident}
    for l, nm_pair in enumerate([("ih0", "hh0"), ("ih1", "hh1")]):
        for nm, Wmat in zip(nm_pair, (W_ih[l], W_hh[l])):
            common[f"w_{nm}"] = wform(Wmat)
        common[f"b{l}"] = np.broadcast_to(
            (b_ih[l] + b_hh[l]).astype(np.float16).reshape(1, H),
            (128, H)).copy()

    in_maps = []
    for c in range(N_CORES):
        bg, j = c // 4, c % 4
        bsl = slice(bg * BG, (bg + 1) * BG)
        osl = slice(j * OS, (j + 1) * OS)
        wfc = np.ascontiguousarray(
            fc_W[osl].T.reshape(KT, 128, OS).transpose(1, 0, 2)).astype(
                np.float16)
        m = dict(common)
        m["w_fc"] = wfc
        m["fcb"] = np.broadcast_to(
            fc_b[osl].astype(np.float16).reshape(1, OS),
            (128, OS)).copy()
        for nm, src in (("x", x[0, bsl]), ("h0", hidden[0, bsl]),
                        ("h1", hidden[1, bsl])):
            m[f"g_{nm}"] = gform(src)
        in_maps.append(m)
    return in_maps


def kernel(x, hidden, embedded, W_ih, W_hh, b_ih, b_hh, fc_W, fc_b,
           _trace=False, _trace_kwargs=None):
    n_steps = embedded.shape[0]
    key = n_steps
    if key not in _cached:
        _cached[key] = _build_program(n_steps)
    nc = _cached[key]

    in_maps = _prep_inputs(np.asarray(x), np.asarray(hidden), np.asarray(W_ih),
                           np.asarray(W_hh), np.asarray(b_ih), np.asarray(b_hh),
                           np.asarray(fc_W), np.asarray(fc_b), n_steps)
    core_ids = list(range(N_CORES))
    res = run_bass_kernel_spmd(nc, in_maps, core_ids, trace=_trace,
                               **(_trace_kwargs or {}))

    out = np.empty((n_steps, 1, B, O), np.float32)
    for c in range(N_CORES):
        bg, j = c // 4, c % 4
        out[:, 0, bg * BG:(bg + 1) * BG, j * OS:(j + 1) * OS] = res.results[c]["out"]
    if _trace:
        kernel.last_results = res
    return out


# revision 8
# speedup vs baseline: 1.0033x; 1.0033x over previous
"""Trainium2 Bass kernel for nn_DecoderRNN: 64-step 2-layer tanh RNN + per-step FC.

Sharding (8 cores, no collectives):
  - 2-way data parallel over batch (cores 0-3: rows 0:128, cores 4-7: rows 128:256).
    Each group of 4 cores redundantly computes its batch-half's RNN.
  - 4-way tensor parallel over the fc output dim (O=8192 -> 2048 per core).

Numerics: everything on the PE runs in fp16 (state, RNN weights, FC weights).
fp16 keeps 10 mantissa bits (eps 4.9e-4) and tanh keeps |h|<=1, so the
recurrent quantization noise stays ~1.2e-2 total (vs 2e-2 gate; bf16 would
be ~7e-2 and fails). fp16 products accumulate exactly into fp32 PSUM.
Why fp16 over the baseline f32r: transposes run 1.0 cycles/row instead of
1.5, DVE ops get 2x throughput, and weight DMA bytes halve.

Per-core compute, per step t (B=128, H=1024, O_slice=2048):
  - State kept transposed ("g" form, [H, B]): g tiles are the matmul stationary
    operand (lhsT), weights stream as the moving operand at N=512.
  - RNN bias is added in-place on PSUM by the DVE (broadcast bias tile) after
    each 512-wide accumulation group, off the PE array. tanh on the scalar
    engine -> fp16 h_sb.
  - Layer outputs ([B, H] in PSUM) are transposed back to g form on the tensor
    engine (fp16 transposes), drained by DVE copies; the same fp16 g_h1 tile
    feeds both the next step's RNN and the FC (no separate cast).
  - FC bias is added during the DVE PSUM drain (broadcast bias tile), and FC
    matmul chunks are interleaved into the tanh/transpose latency gaps of the
    NEXT step to keep the PE array busy.
  - Startup: RNN weights stream on the sync DMA queue in first-needed order
    (layer-0 n-halves first), FC weights on the scalar-engine DMA queue in
    parallel, so step-0 matmuls start ~10us in instead of waiting ~60us for
    the whole weight set.
"""
import sys

sys.path.insert(0, "/opt/trn_rl_repo")

from contextlib import ExitStack

import numpy as np

import concourse.bass as bass
import concourse.tile as tile
from concourse import bacc, mybir
from concourse.bass_utils import run_bass_kernel_spmd

H = 1024
O = 8192
L = 2
T = 64
B = 256
N_CORES = 8
BG = B // 2          # batch rows per core (2-way DP)
OS = O // 4          # fc output slice per core (4-way TP)
KT = H // 128        # 8 k-tiles per 1024 contraction
F32 = mybir.dt.float32
F16 = mybir.dt.float16

_cached = {}

RNN_W_NAMES = ["ih0", "hh0", "ih1", "hh1"]


def _build_program(n_steps: int):
    nc = bacc.Bacc("TRN2", target_bir_lowering=False, debug=False, num_devices=N_CORES)

    # --- DRAM parameters (per-core shards, host-prepared layouts) ---
    # RNN weights, transposed+tiled on host: [p][nck][k][n] of W.T, fp16.
    # The 512-col half (nck) leads the free dims so a half-matrix DMA moves
    # 8KB-contiguous runs per partition (big packets, ~350GB/s).
    wd = {}
    for nm in RNN_W_NAMES:
        wd[nm] = nc.declare_dram_parameter(f"w_{nm}", [128, 2, KT, 512], F16,
                                           isOutput=False)
    w_fc = nc.declare_dram_parameter("w_fc", [128, KT, OS], F16, isOutput=False)
    # initial state, g form: [p][k][b] = state[b, k*128+p], fp16
    gd = {}
    for nm in ("x", "h0", "h1"):
        gd[nm] = nc.declare_dram_parameter(f"g_{nm}", [128, KT, BG], F16,
                                           isOutput=False)
    # bias bcast tiles (b_ih + b_hh per layer), fc bias bcast, identity
    bd = {}
    for l in range(L):
        bd[f"b{l}"] = nc.declare_dram_parameter(f"b{l}", [128, H], F16,
                                                isOutput=False)
    fcbd = nc.declare_dram_parameter("fcb", [128, OS], F16, isOutput=False)
    identd = nc.declare_dram_parameter("ident", [128, 128], F16, isOutput=False)

    out_d = nc.declare_dram_parameter("out", [n_steps, 128, OS], F32, isOutput=True)

    with tile.TileContext(nc) as tc, ExitStack() as ctx:
        wpool = ctx.enter_context(tc.tile_pool(name="w", bufs=1))
        cpool = ctx.enter_context(tc.tile_pool(name="c", bufs=1))
        gp = ctx.enter_context(tc.tile_pool(name="gp", bufs=3))
        hp = ctx.enter_context(tc.tile_pool(name="h", bufs=2))
        logp = ctx.enter_context(tc.tile_pool(name="log", bufs=3))
        rnn_ps = ctx.enter_context(tc.tile_pool(name="rnnps", bufs=1, space="PSUM"))
        tr_ps = ctx.enter_context(tc.tile_pool(name="trps", bufs=2, space="PSUM"))
        fc_ps = ctx.enter_context(tc.tile_pool(name="fcps", bufs=2, space="PSUM"))

        # --- preamble: load constants + weights in first-needed order,
        # split across BOTH HW DGE queues (sync + scalar): smalls first, then
        # layer-0's first 512-col halves (ih on sync, hh on scalar), then the
        # rest; the FC weights trail (not needed until step 1).
        g_tiles = {}
        w = {}
        for nm in RNN_W_NAMES:
            w[nm] = wpool.tile([128, 2, KT, 512], F16, tag=f"w{nm}", name=f"w{nm}")
        for nm in ("x", "h0", "h1"):
            g_tiles[nm] = gp.tile([128, KT, BG], F16, tag="g", name="g")
        ident = cpool.tile([128, 128], F16, tag="ident")
        bb = {}
        for nm in bd:
            bb[nm] = cpool.tile([128, H], F16, tag=nm, name=nm)
        fcb = cpool.tile([128, OS], F16, tag="fcb")
        wfc = wpool.tile([128, KT, OS], F16, tag="wfc")

        nc.sync.dma_start(ident[:], identd[:])
        nc.sync.dma_start(g_tiles["x"][:], gd["x"][:])
        nc.sync.dma_start(g_tiles["h0"][:], gd["h0"][:])
        nc.scalar.dma_start(bb["b0"][:], bd["b0"][:])
        nc.scalar.dma_start(g_tiles["h1"][:], gd["h1"][:])
        nc.scalar.dma_start(bb["b1"][:], bd["b1"][:])
        for nck in range(2):
            nc.sync.dma_start(w["ih0"][:, nck], wd["ih0"][:, nck])
            nc.scalar.dma_start(w["hh0"][:, nck], wd["hh0"][:, nck])
        for nck in range(2):
            nc.sync.dma_start(w["ih1"][:, nck], wd["ih1"][:, nck])
            nc.scalar.dma_start(w["hh1"][:, nck], wd["hh1"][:, nck])
        nc.sync.dma_start(wfc[:], w_fc[:])
        nc.scalar.dma_start(fcb[:], fcbd[:])

        # PE warm-up: ~48 throwaway N=128 matmuls on the identity as soon as
        # it lands. Keeps the HAM activity window busy during the weight DMAs
        # so the real step-0 matmuls run at 2.4GHz instead of 1.2.
        warm_ps = tr_ps.tile([128, 512], F32, tag="trps", name="warm")
        for i in range(48):
            nc.tensor.matmul(warm_ps[:, bass.ts(i % 4, 128)], ident[:], ident[:],
                             start=True, stop=True)

        g_x = g_tiles["x"]
        g_h0 = g_tiles["h0"]
        g_h1 = g_tiles["h1"]

        def rnn_layer(g_in, g_h, w_in, w_h, b):
            """tanh(in @ W_ihT + h @ W_hhT + b) -> h_sb [128(B), H] fp16.

            Bias is added in-place on the DVE (PSUM += bias bcast tile), off
            the PE array; tanh on the scalar engine."""
            ps = rnn_ps.tile([128, H], F32, tag="rnnps")
            h_sb = hp.tile([128, H], F16, tag="h")
            for nck in range(2):
                nsl = bass.ts(nck, 512)
                for pi, (lhs, rhs) in enumerate(((g_in, w_in), (g_h, w_h))):
                    for k in range(KT):
                        first = pi == 0 and k == 0
                        last = pi == 1 and k == KT - 1
                        nc.tensor.matmul(ps[:, nsl], lhs[:, k, :], rhs[:, nck, k, :],
                                         start=first, stop=last)
                nc.vector.tensor_add(ps[:, nsl], ps[:, nsl], b[:, nsl])
                # per-half tanh: the first half runs on the scalar engine
                # while the PE is still on the second half's matmuls
                nc.scalar.activation(h_sb[:, nsl], ps[:, nsl],
                                     mybir.ActivationFunctionType.Tanh)
            return h_sb

        def to_g(h_sb):
            """Transpose [B, H] -> g form [H(p), B] via REGULAR matmuls
            against the identity (out = h_sb_slice.T @ I).

            Regular matmuls pipeline back-to-back through the PE reorder
            window (~100ns each at N=128), unlike transpose-mode whose
            ~170ns fixed SBUF-access latency doesn't overlap. Numerically
            identical: fp16 h values pass through f32 PSUM exactly.
            4 transposed tiles per PSUM bank; each bank drained (and cast
            back to fp16) by a wide DVE copy."""
            g = gp.tile([128, KT, BG], F16, tag="g", name="g")
            for grp in range(2):
                pt = tr_ps.tile([128, 512], F32, tag="trps", name="pt")
                for j in range(4):
                    k = grp * 4 + j
                    nc.tensor.matmul(pt[:, bass.ts(j, 128)],
                                     h_sb[:, bass.ts(k, 128)], ident[:],
                                     start=True, stop=True)
                gs = g[:, grp * 4:(grp + 1) * 4, :]
                nc.vector.tensor_copy(gs, pt[:])
            return g

        def emit_fc_chunk(gb, tprev, ci):
            """FC chunk: logits[:, ci*512:(ci+1)*512] for step tprev (fp16)."""
            ps = fc_ps.tile([128, 512], F32, tag="fcps", name="fps")
            fsl = bass.ts(ci, 512)
            for k in range(KT):
                nc.tensor.matmul(ps[:], gb[:, k, :], wfc[:, k, fsl],
                                 start=(k == 0), stop=(k == KT - 1))
            lsb = logp.tile([128, 512], F32, tag="log", name="lsb")
            nc.vector.tensor_add(lsb[:], ps[:], fcb[:, fsl])
            nc.sync.dma_start(out_d[tprev][:, fsl], lsb[:])

        # Software pipeline: FC of step t-1 is interleaved into step t's
        # tanh/transpose gaps. pending = (g_h1, t_index) awaiting FC.
        pending = (g_h1, None)  # g from init; no FC for it

        for t in range(n_steps):
            gb_prev, tprev = pending
            h0_sb = rnn_layer(g_x, g_h0, w["ih0"], w["hh0"], bb["b0"])
            if tprev is not None:
                emit_fc_chunk(gb_prev, tprev, 0)
            g_h0 = to_g(h0_sb)
            if tprev is not None:
                emit_fc_chunk(gb_prev, tprev, 1)
            h1_sb = rnn_layer(g_h0, g_h1, w["ih1"], w["hh1"], bb["b1"])
            if tprev is not None:
                emit_fc_chunk(gb_prev, tprev, 2)
            g_h1 = to_g(h1_sb)
            if tprev is not None:
                emit_fc_chunk(gb_prev, tprev, 3)
            g_x = g_h1
            pending = (g_h1, t)

        # drain the last step's FC
        gb_prev, tprev = pending
        if tprev is not None:
            for ci in range(4):
                emit_fc_chunk(gb_prev, tprev, ci)

    nc.finalize()
    return nc


def _prep_inputs(x, hidden, W_ih, W_hh, b_ih, b_hh, fc_W, fc_b, n_steps):
    """Build the 8 per-core input maps (host-side transposes, fp16)."""
    def gform(a):  # [BG, H] -> [128, KT, BG]: out[p, k, b] = a[b, k*128+p]
        return np.ascontiguousarray(
            a.T.reshape(KT, 128, BG).transpose(1, 0, 2)).astype(np.float16)

    def wform(Wmat):  # [H_out, H_in] -> [128, 2, KT, 512] of W.T (fp16)
        wt = np.ascontiguousarray(
            Wmat.T.reshape(KT, 128, Wmat.shape[0]).transpose(1, 0, 2)).astype(
                np.float16)  # [128, KT, H_out]
        return np.ascontiguousarray(
            wt.reshape(128, KT, 2, 512).transpose(0, 2, 1, 3))

    ident = np.eye(128, dtype=np.float16)

    common = {"ident": ident}
    for l, nm_pair in enumerate([("ih0", "hh0"), ("ih1", "hh1")]):
        for nm, Wmat in zip(nm_pair, (W_ih[l], W_hh[l])):
            common[f"w_{nm}"] = wform(Wmat)
        common[f"b{l}"] = np.broadcast_to(
            (b_ih[l] + b_hh[l]).astype(np.float16).reshape(1, H),
            (128, H)).copy()

    in_maps = []
    for c in range(N_CORES):
        bg, j = c // 4, c % 4
        bsl = slice(bg * BG, (bg + 1) * BG)
        osl = slice(j * OS, (j + 1) * OS)
        wfc = np.ascontiguousarray(
            fc_W[osl].T.reshape(KT, 128, OS).transpose(1, 0, 2)).astype(
                np.float16)
        m = dict(common)
        m["w_fc"] = wfc
        m["fcb"] = np.broadcast_to(
            fc_b[osl].astype(np.float16).reshape(1, OS),
            (128, OS)).copy()
        for nm, src in (("x", x[0, bsl]), ("h0", hidden[0, bsl]),
                        ("h1", hidden[1, bsl])):
            m[f"g_{nm}"] = gform(src)
        in_maps.append(m)
    return in_maps


def kernel(x, hidden, embedded, W_ih, W_hh, b_ih, b_hh, fc_W, fc_b,
           _trace=False, _trace_kwargs=None):
    n_steps = embedded.shape[0]
    key = n_steps
    if key not in _cached:
        _cached[key] = _build_program(n_steps)
    nc = _cached[key]

    in_maps = _prep_inputs(np.asarray(x), np.asarray(hidden), np.asarray(W_ih),
                           np.asarray(W_hh), np.asarray(b_ih), np.asarray(b_hh),
                           np.asarray(fc_W), np.asarray(fc_b), n_steps)
    core_ids = list(range(N_CORES))
    res = run_bass_kernel_spmd(nc, in_maps, core_ids, trace=_trace,
                               **(_trace_kwargs or {}))

    out = np.empty((n_steps, 1, B, O), np.float32)
    for c in range(N_CORES):
        bg, j = c // 4, c % 4
        out[:, 0, bg * BG:(bg + 1) * BG, j * OS:(j + 1) * OS] = res.results[c]["out"]
    if _trace:
        kernel.last_results = res
    return out
